# revision 3
# baseline (speedup 1.0000x reference)
"""Trainium2 Bass kernel for AttentionReadoutAtom (global-softmax segment reduce).

Math:  scores = x @ w + b ; attn = softmax(scores over all N) ;
       out[s] = sum_{i: label_i = s} attn_i * x_i          -> [50000, 128]

Softmax is shift/scale invariant: exp(score) without max-subtraction is safe
here (scores ~ N(0,1)), and the bias b cancels between numerator and
denominator.  Using xw = x * w (host-side sharding layout prep):

    out[s, d] = sum_{i in s} e_i * xw_i[d] / (w[d] * Z),   Z = sum_i e_i

Sharding (host, inside kernel()):
  * Sort rows by segment label; greedily pack whole segments into blocks of
    1024 rows (8 tiles of 128 rows) covering <= 128 distinct segments each;
    pad each block to 1024 rows with zero rows.  Every segment lives in
    exactly one block -> no cross-core combination of outputs is needed;
    the only global quantity is the softmax denominator Z, reduced on the
    host from per-core partial e sums (the hint's denominator all-reduce).
  * Blocks are dealt contiguously to 8 cores, padded to equal count B.
  * xw is shipped bf16 (FAST) or as a bf16 hi/lo pair (SPLIT, default),
    pre-arranged [B, half, 128, 1024] so every DMA is a contiguous 2KB/row
    super-tile.

Device per row-tile t of a block (Tile framework schedules all engines):
  * score[p] = sum_d xw[t*128+p, d]     (DVE tensor_scalar accum_out, with
               some row-tiles' score op placed on ScalarE to balance engines)
  * e = exp(score)                      (ScalarE, one op per 8-block chunk;
               e chunk is DMA'd out for the host-side Z reduction)
  * Me[p, s] = (iota[s] == lab_rel[p]) * e[p]   (one DVE tensor_scalar,
               dual-ALU: is_equal then mult with two [P,1] operands)
  * psum[s, d] += Me^T @ xw_tile        (TensorE, PSUM accumulation over the
               block; SPLIT mode issues Mh@xh + Mh@xl + Ml@xh, which
               reconstructs the f32 product to ~4e-5 because bf16*bf16
               products are exact in the PE's f32 accumulation)
  * evict psum -> SBUF -> DRAM          (ScalarE copy + DMA)

Host epilogue: scatter per-block rows to the full [50000, 128] output and
apply the scalar normalization out / (w[d] * Z).
"""

import os
import numpy as np
import ml_dtypes

# ---------------------------------------------------------------- constants
N = 500000
D = 128
NUM_SEGMENTS = 50000
N_CORES = 8
P = 128
TPB = 8                   # row tiles per block
ROWS_PER_BLOCK = TPB * P  # 1024
MAX_SEGS_PER_BLOCK = 128
CHUNK_BLOCKS = 8          # blocks per e/lab chunk (64 row tiles)

MODE = os.environ.get("ATTN_KERNEL_MODE", "split")  # "split" | "fast"
# number of leading blocks per chunk whose score op runs on DVE (rest: ScalarE)
DVE_SCORE_BLOCKS = {"split": 3, "fast": 5}

_COMPILED = {}


# ---------------------------------------------------------------- device code
def _build_kernel(B, mode):
    import concourse.bacc as bacc
    import concourse.mybir as mybir
    from concourse.tile import TileContext

    f32 = mybir.dt.float32
    bf16 = mybir.dt.bfloat16
    Alu = mybir.AluOpType
    Act = mybir.ActivationFunctionType

    nsplit = 2 if mode == "split" else 1
    NT = B * TPB
    NCHUNK = (B + CHUNK_BLOCKS - 1) // CHUNK_BLOCKS
    CC = CHUNK_BLOCKS * TPB          # score/e columns per chunk
    dve_blocks = DVE_SCORE_BLOCKS[mode]

    nc = bacc.Bacc("TRN2", target_bir_lowering=False, debug=False,
                   num_devices=N_CORES)

    xw_d = nc.dram_tensor("xw", [B, nsplit, P, TPB * P], bf16,
                          kind="ExternalInput")
    lab_d = nc.dram_tensor("lab", [NCHUNK, P, CC], f32, kind="ExternalInput")
    out_d = nc.dram_tensor("out", [B, P, P], f32, kind="ExternalOutput")
    z_d = nc.dram_tensor("zpart", [NCHUNK, P, CC], f32, kind="ExternalOutput")

    with TileContext(nc) as tc:
        with tc.tile_pool(name="const", bufs=1) as cpool, \
             tc.tile_pool(name="xwp", bufs=2 * CHUNK_BLOCKS) as xwp, \
             tc.tile_pool(name="labp", bufs=2) as labp, \
             tc.tile_pool(name="scp", bufs=2) as scp, \
             tc.tile_pool(name="mep", bufs=4) as mep, \
             tc.tile_pool(name="evp", bufs=3) as evp, \
             tc.tile_pool(name="psum", bufs=4, space="PSUM") as psp:

            iota_i = cpool.tile([P, P], mybir.dt.int32)
            nc.gpsimd.iota(iota_i[:], pattern=[[1, P]], base=0,
                           channel_multiplier=0)
            iota_b = cpool.tile([P, P], bf16)
            nc.vector.tensor_copy(iota_b[:], iota_i[:])

            for ch in range(NCHUNK):
                blocks = list(range(ch * CHUNK_BLOCKS,
                                    min((ch + 1) * CHUNK_BLOCKS, B)))
                nb = len(blocks)
                ntile = nb * TPB
                n_dve = min(dve_blocks, nb) * TPB   # score cols on DVE

                lab_t = labp.tile([P, CC], f32, tag="lab")
                nc.sync.dma_start(lab_t[:, :ntile], lab_d.ap()[ch, :, :ntile])

                sc_dve = scp.tile([P, CC], f32, tag="sc_dve")
                sc_act = scp.tile([P, CC], f32, tag="sc_act")
                e_t = scp.tile([P, CC], f32, tag="e")
                if mode == "split":
                    ehib_t = scp.tile([P, CC], bf16, tag="ehib")
                    elo_t = scp.tile([P, CC], f32, tag="elo")
                junk_d = scp.tile([P, nsplit * P], bf16, tag="junk_d")
                junk_a = scp.tile([P, nsplit * P], bf16, tag="junk_a")

                xw_tiles = []
                for bi, b in enumerate(blocks):
                    xw_t = xwp.tile([P, nsplit * TPB * P], bf16, tag="xw")
                    for h in range(nsplit):
                        nc.sync.dma_start(
                            xw_t[:, h * TPB * P:(h + 1) * TPB * P],
                            xw_d.ap()[b, h, :, :])
                    xw_tiles.append(xw_t)
                    xw3 = xw_t[:].rearrange("p (h td) -> p h td", h=nsplit)
                    jd3 = junk_d[:].rearrange("p (h d) -> p h d", h=nsplit)
                    ja3 = junk_a[:].rearrange("p (h d) -> p h d", h=nsplit)
                    for t in range(TPB):
                        col = bi * TPB + t
                        src = xw3[:, :, t * P:(t + 1) * P]   # [P, nsplit, 128]
                        if col < n_dve:
                            nc.vector.tensor_scalar(
                                out=jd3[:, :, :], in0=src,
                                scalar1=1.0, scalar2=0.0,
                                op0=Alu.mult, op1=Alu.add,
                                accum_out=sc_dve[:, col:col + 1])
                        else:
                            nc.scalar.activation(
                                out=ja3[:, :, :], in_=src, func=Act.Copy,
                                accum_out=sc_act[:, col:col + 1])

                # e = exp(score): one ACT op per score tile region
                if n_dve > 0:
                    nc.scalar.activation(out=e_t[:, :n_dve],
                                         in_=sc_dve[:, :n_dve], func=Act.Exp)
                if ntile > n_dve:
                    nc.scalar.activation(out=e_t[:, n_dve:ntile],
                                         in_=sc_act[:, n_dve:ntile],
                                         func=Act.Exp)
                # ship e for the host-side Z reduction
                nc.sync.dma_start(z_d.ap()[ch, :, :ntile], e_t[:, :ntile])
                if mode == "split":
                    nc.vector.tensor_copy(ehib_t[:, :ntile], e_t[:, :ntile])
                    nc.vector.tensor_tensor(
                        out=elo_t[:, :ntile], in0=e_t[:, :ntile],
                        in1=ehib_t[:, :ntile], op=Alu.subtract)

                for bi, b in enumerate(blocks):
                    xw_t = xw_tiles[bi]
                    ps = psp.tile([P, P], f32, tag="acc")
                    n_mm = 3 * TPB if mode == "split" else TPB
                    mm = 0
                    for t in range(TPB):
                        col = bi * TPB + t
                        me_h = mep.tile([P, P], bf16, tag="meh")
                        nc.vector.tensor_scalar(
                            out=me_h[:], in0=iota_b[:],
                            scalar1=lab_t[:, col:col + 1],
                            scalar2=e_t[:, col:col + 1],
                            op0=Alu.is_equal, op1=Alu.mult)
                        xh = xw_t[:, t * P:(t + 1) * P]
                        nc.tensor.matmul(ps[:], lhsT=me_h[:], rhs=xh,
                                         start=(mm == 0),
                                         stop=(mm == n_mm - 1))
                        mm += 1
                        if mode == "split":
                            xl = xw_t[:, (TPB + t) * P:(TPB + t + 1) * P]
                            nc.tensor.matmul(ps[:], lhsT=me_h[:], rhs=xl,
                                             start=False,
                                             stop=(mm == n_mm - 1))
                            mm += 1
                            me_l = mep.tile([P, P], bf16, tag="mel")
                            nc.vector.tensor_scalar(
                                out=me_l[:], in0=iota_b[:],
                                scalar1=lab_t[:, col:col + 1],
                                scalar2=elo_t[:, col:col + 1],
                                op0=Alu.is_equal, op1=Alu.mult)
                            nc.tensor.matmul(ps[:], lhsT=me_l[:], rhs=xh,
                                             start=False,
                                             stop=(mm == n_mm - 1))
                            mm += 1
                    ev = evp.tile([P, P], f32, tag="ev")
                    nc.scalar.copy(ev[:], ps[:])
                    nc.sync.dma_start(out_d.ap()[b, :, :], ev[:])

    nc.compile()
    return nc


# ---------------------------------------------------------------- host side
def _pack_blocks(counts):
    blocks = []
    s, nseg = 0, len(counts)
    while s < nseg:
        rows, s0 = 0, s
        while s < nseg and s - s0 < MAX_SEGS_PER_BLOCK:
            c = counts[s]
            if rows + c > ROWS_PER_BLOCK:
                break
            rows += int(c)
            s += 1
        assert s > s0, f"segment {s0} with {counts[s0]} rows exceeds a block"
        blocks.append((s0, s, rows))
    return blocks


def _numpy_fallback(x, labels, w, b):
    scores = x.astype(np.float64) @ w.astype(np.float64) + float(b)
    scores -= scores.max()
    e = np.exp(scores)
    a = e / e.sum()
    out = np.zeros((NUM_SEGMENTS, x.shape[1]), np.float64)
    np.add.at(out, labels, x * a[:, None])
    return out.astype(np.float32)


def kernel(x, monomer_labels_i, attn_w, attn_b):
    from concourse import bass_utils

    x = np.ascontiguousarray(np.asarray(x, dtype=np.float32))
    labels = np.asarray(monomer_labels_i).astype(np.int64)
    w = np.asarray(attn_w, dtype=np.float32)
    b = np.float32(np.asarray(attn_b))

    if np.abs(w).min() < 1e-30 or np.bincount(
            labels, minlength=NUM_SEGMENTS).max() > ROWS_PER_BLOCK:
        return _numpy_fallback(x, labels, w, b)

    order = np.argsort(labels, kind="stable")
    labels_s = labels[order]
    counts = np.bincount(labels, minlength=NUM_SEGMENTS)
    blocks = _pack_blocks(counts)
    nblocks = len(blocks)
    B = (nblocks + N_CORES - 1) // N_CORES
    NCHUNK = (B + CHUNK_BLOCKS - 1) // CHUNK_BLOCKS
    CC = CHUNK_BLOCKS * TPB
    seg_row_start = np.zeros(NUM_SEGMENTS + 1, np.int64)
    np.cumsum(counts, out=seg_row_start[1:])

    nsplit = 2 if MODE == "split" else 1
    xw = x[order] * w[None, :]
    xw_hi = xw.astype(ml_dtypes.bfloat16)
    if MODE == "split":
        xw_lo = (xw - xw_hi.astype(np.float32)).astype(ml_dtypes.bfloat16)

    in_maps = []
    meta = []
    n_pad_rows = 0
    for c in range(N_CORES):
        xw_dev = np.zeros((B, nsplit, P, TPB * P), ml_dtypes.bfloat16)
        lab_dev = np.full((NCHUNK, P, CC), 127.0, np.float32)
        meta_c = []
        for bi in range(B):
            gi = c * B + bi
            if gi >= nblocks:
                meta_c.append(None)
                n_pad_rows += ROWS_PER_BLOCK
                continue
            s0, s1, rows = blocks[gi]
            r0 = seg_row_start[s0]
            ch, pos = divmod(bi, CHUNK_BLOCKS)

            def pack(src_rows):
                full = np.zeros((ROWS_PER_BLOCK, D), src_rows.dtype)
                full[:rows] = src_rows
                return full.reshape(TPB, P, D).transpose(1, 0, 2).reshape(
                    P, TPB * P)

            xw_dev[bi, 0] = pack(xw_hi[r0:r0 + rows])
            if MODE == "split":
                xw_dev[bi, 1] = pack(xw_lo[r0:r0 + rows])
            fl = np.full(ROWS_PER_BLOCK, 127.0, np.float32)
            fl[:rows] = (labels_s[r0:r0 + rows] - s0).astype(np.float32)
            lab_dev[ch, :, pos * TPB:(pos + 1) * TPB] = \
                fl.reshape(TPB, P).transpose(1, 0)
            n_pad_rows += ROWS_PER_BLOCK - rows
            meta_c.append((int(s0), int(s1)))
        meta.append(meta_c)
        in_maps.append({"xw": xw_dev, "lab": lab_dev})

    key = (B, MODE)
    if key not in _COMPILED:
        _COMPILED[key] = _build_kernel(B, MODE)
    nc = _COMPILED[key]

    res = bass_utils.run_bass_kernel_spmd(nc, in_maps,
                                          core_ids=list(range(N_CORES)))

    # ---- gather / unshard
    Z = 0.0
    out = np.zeros((NUM_SEGMENTS, D), np.float32)
    for c in range(N_CORES):
        r = res.results[c]
        zp = r["zpart"]
        for ch in range(NCHUNK):
            ntile = (min((ch + 1) * CHUNK_BLOCKS, B) - ch * CHUNK_BLOCKS) * TPB
            Z += float(zp[ch, :, :ntile].astype(np.float64).sum())
        out_dev = r["out"]
        for bi in range(B):
            m = meta[c][bi]
            if m is None:
                continue
            s0, s1 = m
            out[s0:s1] = out_dev[bi, :s1 - s0, :]
    # pad rows have xw == 0 -> score 0 -> e = exp(0) = 1 each
    Z -= float(n_pad_rows)
    out /= (w[None, :] * np.float32(Z))
    return out.astype(np.float32)


if __name__ == "__main__":
    from ref_io import get
    inputs, expected = get()
    out = kernel(**inputs)
    err = np.abs(out - expected)
    print("absmax err:", err.max(), "scale-rel:",
          err.max() / np.abs(expected).max())


# revision 6
# speedup vs baseline: 1.9654x; 1.9654x over previous
"""Trainium2 Bass kernel for AttentionReadoutAtom (global-softmax segment reduce).

Math:  scores = x @ w + b ; attn = softmax(scores over all N) ;
       out[s] = sum_{i: label_i = s} attn_i * x_i          -> [50000, 128]

Softmax is shift/scale invariant: exp(score) without max-subtraction is safe
here (scores ~ N(0,1)), and the bias b cancels between numerator and
denominator.  Using xw = x * w (host-side sharding layout prep):

    out[s, d] = sum_{i in s} e_i * xw_i[d] / (w[d] * Z),   Z = sum_i e_i

Sharding (host, inside kernel()):
  * Sort rows by segment label; greedily pack whole segments into blocks of
    1024 rows (8 tiles of 128 rows) covering <= 128 distinct segments each;
    pad each block to 1024 rows with zero rows.  Every segment lives in
    exactly one block -> no cross-core combination of outputs is needed;
    the only global quantity is the softmax denominator Z, reduced on the
    host from per-core partial e sums (the hint's denominator all-reduce).
  * Blocks are dealt contiguously to 8 cores, padded to equal count B.
  * xw is shipped bf16 (FAST) or as a bf16 hi/lo pair (SPLIT, default),
    pre-arranged [B, half, 128, 1024] so every DMA is a contiguous 2KB/row
    super-tile.

Device per row-tile t of a block (Tile framework schedules all engines):
  * score[p] = sum_d xw[t*128+p, d]     (DVE tensor_scalar accum_out, with
               some row-tiles' score op placed on ScalarE to balance engines)
  * e = exp(score)                      (ScalarE, one op per 8-block chunk;
               e chunk is DMA'd out for the host-side Z reduction)
  * Me[p, s] = (iota[s] == lab_rel[p]) * e[p]   (one DVE tensor_scalar,
               dual-ALU: is_equal then mult with two [P,1] operands)
  * psum[s, d] += Me^T @ xw_tile        (TensorE, PSUM accumulation over the
               block; SPLIT mode issues Mh@xh + Mh@xl + Ml@xh, which
               reconstructs the f32 product to ~4e-5 because bf16*bf16
               products are exact in the PE's f32 accumulation)
  * evict psum -> SBUF -> DRAM          (ScalarE copy + DMA)

Host epilogue: scatter per-block rows to the full [50000, 128] output and
apply the scalar normalization out / (w[d] * Z).
"""

import os
import numpy as np
import ml_dtypes

# ---------------------------------------------------------------- constants
N = 500000
D = 128
NUM_SEGMENTS = 50000
N_CORES = 8
P = 128
TPB = 8                   # row tiles per block
ROWS_PER_BLOCK = TPB * P  # 1024
MAX_SEGS_PER_BLOCK = 128
CHUNK_BLOCKS = 8          # blocks per e/lab chunk (64 row tiles)

MODE = os.environ.get("ATTN_KERNEL_MODE", "split")  # "split" | "fast"
# number of leading blocks per chunk whose score op runs on DVE (rest: ScalarE)
DVE_SCORE_MOD = {"split": 4, "fast": 2}  # every k-th row-tile's score on DVE (must divide TPB)

_COMPILED = {}


# ---------------------------------------------------------------- device code
def _build_kernel(B, mode):
    import concourse.bacc as bacc
    import concourse.mybir as mybir
    from concourse.tile import TileContext

    f32 = mybir.dt.float32
    bf16 = mybir.dt.bfloat16
    Alu = mybir.AluOpType
    Act = mybir.ActivationFunctionType

    nsplit = 2 if mode == "split" else 1
    NT = B * TPB
    NCHUNK = (B + CHUNK_BLOCKS - 1) // CHUNK_BLOCKS
    CC = CHUNK_BLOCKS * TPB          # score/e columns per chunk
    dve_mod = DVE_SCORE_MOD[mode]

    nc = bacc.Bacc("TRN2", target_bir_lowering=False, debug=False,
                   num_devices=N_CORES)

    xw_d = nc.dram_tensor("xw", [B, nsplit, P, TPB * P], bf16,
                          kind="ExternalInput")
    lab_d = nc.dram_tensor("lab", [NCHUNK, P, CC], f32, kind="ExternalInput")
    out_d = nc.dram_tensor("out", [B, P, P], f32, kind="ExternalOutput")
    z_d = nc.dram_tensor("zpart", [NCHUNK, P, CC], f32, kind="ExternalOutput")

    with TileContext(nc) as tc:
        with tc.tile_pool(name="const", bufs=1) as cpool, \
             tc.tile_pool(name="xwp", bufs=2 * CHUNK_BLOCKS) as xwp, \
             tc.tile_pool(name="labp", bufs=2) as labp, \
             tc.tile_pool(name="scp", bufs=2) as scp, \
             tc.tile_pool(name="mep", bufs=8) as mep, \
             tc.tile_pool(name="evp", bufs=3) as evp, \
             tc.tile_pool(name="psum", bufs=6, space="PSUM") as psp:

            iota_i = cpool.tile([P, P], mybir.dt.int32)
            nc.gpsimd.iota(iota_i[:], pattern=[[1, P]], base=0,
                           channel_multiplier=0)
            iota_b = cpool.tile([P, P], bf16)
            nc.vector.tensor_copy(iota_b[:], iota_i[:])

            for ch in range(NCHUNK):
                blocks = list(range(ch * CHUNK_BLOCKS,
                                    min((ch + 1) * CHUNK_BLOCKS, B)))
                nb = len(blocks)
                ntile = nb * TPB

                lab_t = labp.tile([P, CC], f32, tag="lab")
                nc.sync.dma_start(lab_t[:, :ntile], lab_d.ap()[ch, :, :ntile])

                sc_dve = scp.tile([P, CC], f32, tag="sc_dve")
                sc_act = scp.tile([P, CC], f32, tag="sc_act")
                e_t = scp.tile([P, CC], f32, tag="e")
                if mode == "split":
                    ehib_t = scp.tile([P, CC], bf16, tag="ehib")
                    elo_t = scp.tile([P, CC], f32, tag="elo")
                junk_d = scp.tile([P, nsplit * P], bf16, tag="junk_d")
                junk_a = scp.tile([P, nsplit * P], bf16, tag="junk_a")

                xw_tiles = []
                for bi, b in enumerate(blocks):
                    xw_t = xwp.tile([P, nsplit * TPB * P], bf16, tag="xw")
                    xw3 = xw_t[:].rearrange("p (h td) -> p h td", h=nsplit)
                    nc.sync.dma_start(
                        xw3[:, :, :],
                        xw_d.ap()[b].rearrange("h p c -> p h c"))
                    xw_tiles.append(xw_t)
                    jd3 = junk_d[:].rearrange("p (h d) -> p h d", h=nsplit)
                    ja3 = junk_a[:].rearrange("p (h d) -> p h d", h=nsplit)
                    for t in range(TPB):
                        col = bi * TPB + t
                        src = xw3[:, :, t * P:(t + 1) * P]   # [P, nsplit, 128]
                        if col % dve_mod == 0:
                            nc.vector.tensor_scalar(
                                out=jd3[:, :, :], in0=src,
                                scalar1=1.0, scalar2=0.0,
                                op0=Alu.mult, op1=Alu.add,
                                accum_out=sc_dve[:, col:col + 1])
                        else:
                            nc.scalar.activation(
                                out=ja3[:, :, :], in_=src, func=Act.Copy,
                                accum_out=sc_act[:, col:col + 1])

                # e = exp(score): strided views select each engine's columns
                sc3d = sc_dve[:].rearrange("p (g k) -> p g k", k=dve_mod)
                sa3d = sc_act[:].rearrange("p (g k) -> p g k", k=dve_mod)
                e3d = e_t[:].rearrange("p (g k) -> p g k", k=dve_mod)
                ng = ntile // dve_mod
                nc.scalar.activation(out=e3d[:, :ng, 0:1],
                                     in_=sc3d[:, :ng, 0:1], func=Act.Exp)
                nc.scalar.activation(out=e3d[:, :ng, 1:dve_mod],
                                     in_=sa3d[:, :ng, 1:dve_mod], func=Act.Exp)
                # ship e for the host-side Z reduction (pads in the last
                # group of a short chunk were never written: zero them via
                # host-side masking instead -> here just DMA what exists)
                nc.sync.dma_start(z_d.ap()[ch, :, :ntile], e_t[:, :ntile])
                if mode == "split":
                    nc.vector.tensor_copy(ehib_t[:, :ntile], e_t[:, :ntile])
                    nc.vector.tensor_tensor(
                        out=elo_t[:, :ntile], in0=e_t[:, :ntile],
                        in1=ehib_t[:, :ntile], op=Alu.subtract)

                for bi, b in enumerate(blocks):
                    xw_t = xw_tiles[bi]
                    ps = psp.tile([P, P], f32, tag="acc")
                    n_mm = 3 * TPB if mode == "split" else TPB
                    mm = 0
                    for t in range(TPB):
                        col = bi * TPB + t
                        me_h = mep.tile([P, P], bf16, tag="meh")
                        nc.vector.tensor_scalar(
                            out=me_h[:], in0=iota_b[:],
                            scalar1=lab_t[:, col:col + 1],
                            scalar2=e_t[:, col:col + 1],
                            op0=Alu.is_equal, op1=Alu.mult)
                        xh = xw_t[:, t * P:(t + 1) * P]
                        nc.tensor.matmul(ps[:], lhsT=me_h[:], rhs=xh,
                                         start=(mm == 0),
                                         stop=(mm == n_mm - 1))
                        mm += 1
                        if mode == "split":
                            xl = xw_t[:, (TPB + t) * P:(TPB + t + 1) * P]
                            nc.tensor.matmul(ps[:], lhsT=me_h[:], rhs=xl,
                                             start=False,
                                             stop=(mm == n_mm - 1))
                            mm += 1
                            me_l = mep.tile([P, P], bf16, tag="mel")
                            nc.vector.tensor_scalar(
                                out=me_l[:], in0=iota_b[:],
                                scalar1=lab_t[:, col:col + 1],
                                scalar2=elo_t[:, col:col + 1],
                                op0=Alu.is_equal, op1=Alu.mult)
                            nc.tensor.matmul(ps[:], lhsT=me_l[:], rhs=xh,
                                             start=False,
                                             stop=(mm == n_mm - 1))
                            mm += 1
                    ev = evp.tile([P, P], f32, tag="ev")
                    nc.scalar.copy(ev[:], ps[:])
                    nc.sync.dma_start(out_d.ap()[b, :, :], ev[:])

    nc.compile()
    return nc


# ---------------------------------------------------------------- host side
def _pack_blocks(counts):
    blocks = []
    s, nseg = 0, len(counts)
    while s < nseg:
        rows, s0 = 0, s
        while s < nseg and s - s0 < MAX_SEGS_PER_BLOCK:
            c = counts[s]
            if rows + c > ROWS_PER_BLOCK:
                break
            rows += int(c)
            s += 1
        assert s > s0, f"segment {s0} with {counts[s0]} rows exceeds a block"
        blocks.append((s0, s, rows))
    return blocks


def _numpy_fallback(x, labels, w, b):
    scores = x.astype(np.float64) @ w.astype(np.float64) + float(b)
    scores -= scores.max()
    e = np.exp(scores)
    a = e / e.sum()
    out = np.zeros((NUM_SEGMENTS, x.shape[1]), np.float64)
    np.add.at(out, labels, x * a[:, None])
    return out.astype(np.float32)


def kernel(x, monomer_labels_i, attn_w, attn_b):
    from concourse import bass_utils

    x = np.ascontiguousarray(np.asarray(x, dtype=np.float32))
    labels = np.asarray(monomer_labels_i).astype(np.int64)
    w = np.asarray(attn_w, dtype=np.float32)
    b = np.float32(np.asarray(attn_b))

    if np.abs(w).min() < 1e-30 or np.bincount(
            labels, minlength=NUM_SEGMENTS).max() > ROWS_PER_BLOCK:
        return _numpy_fallback(x, labels, w, b)

    order = np.argsort(labels, kind="stable")
    labels_s = labels[order]
    counts = np.bincount(labels, minlength=NUM_SEGMENTS)
    blocks = _pack_blocks(counts)
    nblocks = len(blocks)
    B = (nblocks + N_CORES - 1) // N_CORES
    NCHUNK = (B + CHUNK_BLOCKS - 1) // CHUNK_BLOCKS
    CC = CHUNK_BLOCKS * TPB
    seg_row_start = np.zeros(NUM_SEGMENTS + 1, np.int64)
    np.cumsum(counts, out=seg_row_start[1:])

    nsplit = 2 if MODE == "split" else 1
    xw = x[order] * w[None, :]
    xw_hi = xw.astype(ml_dtypes.bfloat16)
    if MODE == "split":
        xw_lo = (xw - xw_hi.astype(np.float32)).astype(ml_dtypes.bfloat16)

    in_maps = []
    meta = []
    n_pad_rows = 0
    for c in range(N_CORES):
        xw_dev = np.zeros((B, nsplit, P, TPB * P), ml_dtypes.bfloat16)
        lab_dev = np.full((NCHUNK, P, CC), 127.0, np.float32)
        meta_c = []
        for bi in range(B):
            gi = c * B + bi
            if gi >= nblocks:
                meta_c.append(None)
                n_pad_rows += ROWS_PER_BLOCK
                continue
            s0, s1, rows = blocks[gi]
            r0 = seg_row_start[s0]
            ch, pos = divmod(bi, CHUNK_BLOCKS)

            def pack(src_rows):
                full = np.zeros((ROWS_PER_BLOCK, D), src_rows.dtype)
                full[:rows] = src_rows
                return full.reshape(TPB, P, D).transpose(1, 0, 2).reshape(
                    P, TPB * P)

            xw_dev[bi, 0] = pack(xw_hi[r0:r0 + rows])
            if MODE == "split":
                xw_dev[bi, 1] = pack(xw_lo[r0:r0 + rows])
            fl = np.full(ROWS_PER_BLOCK, 127.0, np.float32)
            fl[:rows] = (labels_s[r0:r0 + rows] - s0).astype(np.float32)
            lab_dev[ch, :, pos * TPB:(pos + 1) * TPB] = \
                fl.reshape(TPB, P).transpose(1, 0)
            n_pad_rows += ROWS_PER_BLOCK - rows
            meta_c.append((int(s0), int(s1)))
        meta.append(meta_c)
        in_maps.append({"xw": xw_dev, "lab": lab_dev})

    key = (B, MODE)
    if key not in _COMPILED:
        _COMPILED[key] = _build_kernel(B, MODE)
    nc = _COMPILED[key]

    res = bass_utils.run_bass_kernel_spmd(nc, in_maps,
                                          core_ids=list(range(N_CORES)))

    # ---- gather / unshard
    Z = 0.0
    out = np.zeros((NUM_SEGMENTS, D), np.float32)
    for c in range(N_CORES):
        r = res.results[c]
        zp = r["zpart"]
        for ch in range(NCHUNK):
            ntile = (min((ch + 1) * CHUNK_BLOCKS, B) - ch * CHUNK_BLOCKS) * TPB
            Z += float(zp[ch, :, :ntile].astype(np.float64).sum())
        out_dev = r["out"]
        for bi in range(B):
            m = meta[c][bi]
            if m is None:
                continue
            s0, s1 = m
            out[s0:s1] = out_dev[bi, :s1 - s0, :]
    # pad rows have xw == 0 -> score 0 -> e = exp(0) = 1 each
    Z -= float(n_pad_rows)
    out /= (w[None, :] * np.float32(Z))
    return out.astype(np.float32)


if __name__ == "__main__":
    from ref_io import get
    inputs, expected = get()
    out = kernel(**inputs)
    err = np.abs(out - expected)
    print("absmax err:", err.max(), "scale-rel:",
          err.max() / np.abs(expected).max())
